# revision 27
# baseline (speedup 1.0000x reference)
"""Trainium2 Bass kernel for the DagnabbitAutoEncoder DAG scan.

Strategy: level-scheduled scan (node depth = 1 + max parent depth; ~28
levels), type-sharded across the 8 NeuronCores (type t -> cores 2t, 2t+1),
fp16 datapath with fp32 PSUM accumulation.

v2 improvements over the AllGather-per-level baseline:
  * parent-swap: every node has >=1 parent in the immediately previous
    level.  Nodes whose OTHER parent is old get their parent slots swapped
    (using a row-permuted copy of W1) so gather slot 0 always reads an old
    row.  All slot-0 gather columns + their PE transposes are then
    independent of the previous AllGather and run in its shadow.
  * read-packing: rows never referenced by any later node are placed after
    the AllGather span and are not exchanged at all; together with 32-row
    padding this cuts AllGather bytes ~40%.
  * the second MLP layer runs with lhsT = h (feature-major hidden) and
    rhs = W2, producing the node-major output tile directly in PSUM - no
    output PE transposes, and the PSUM->SBUF eviction fuses the b2 add.
  * gather columns are grouped [slot0 | slot1] per level and issued as a
    few fused multi-column indirect DMAs instead of one DMA per column.
  * per-tile output DMAs into the collective staging buffer overlap the
    remaining compute; the last level (no future readers) skips its
    AllGather entirely.

The host merges the 8 partial outputs (each core owns its shard rows).
"""

import math
import os

import numpy as np

R = 256
D = 256
NCORES = 8
P = 128
PSUM_N = 512
AGPAD = 32      # AllGather span padding granularity (rows)
SWAP_MIN = int(os.environ.get("DAG_SWAP_MIN", "2048"))
USE_DG = int(os.environ.get("DAG_USE_DG", "0"))


# ---------------------------------------------------------------------------
# host-side preprocessing
# ---------------------------------------------------------------------------

def _compute_levels(idx):
    n = idx.shape[0]
    depth = np.zeros(R + n, np.int32)
    ia = idx[:, 0]
    ib = idx[:, 1]
    d = depth
    for i in range(n):
        da = d[ia[i]]
        db = d[ib[i]]
        d[R + i] = (da if da > db else db) + 1
    return depth[R:]


def _plan(idx, types):
    n = idx.shape[0]
    lv = _compute_levels(idx)
    L = int(lv.max()) if n else 0
    rowlv = np.zeros(R + n, np.int64)
    rowlv[R:] = lv
    read = np.zeros(R + n, bool)
    read[idx.ravel()] = True

    order = np.argsort(lv, kind="stable")
    lv_sorted = lv[order]
    level_nodes = []
    lo = 0
    for l in range(1, L + 1):
        hi = lo + np.searchsorted(lv_sorted[lo:], l + 1)
        level_nodes.append(order[lo:hi])
        lo = hi

    pos = np.zeros(R + n, np.int64)
    pos[:R] = np.arange(R)
    swap = np.zeros(n, bool)

    blk = R
    blk_starts = []
    specs = []       # per level: k, npad, agb, ranges
    shards = []      # per level: per-core (node_ids, slots)
    for l0 in range(L):
        l = l0 + 1
        nodes = level_nodes[l0]
        blk_starts.append(blk)
        per_core = []
        for t in range(4):
            nt = nodes[types[nodes] == t]
            per_core.append(nt[0::2])
            per_core.append(nt[1::2])
        # the parent-swap trick pays for its extra weight ranges only on
        # levels with substantial per-core shards
        use_swap = len(nodes) > SWAP_MIN
        chunks = []   # per core: (Ar, Cr, Br, Bu, Au, Cu) node-id arrays
        for c in range(NCORES):
            s = per_core[c]
            if l > 1:
                p0new = rowlv[idx[s, 0]] == l - 1
                p1new = rowlv[idx[s, 1]] == l - 1
            else:
                p0new = np.zeros(len(s), bool)
                p1new = np.zeros(len(s), bool)
            if use_swap:
                bmask = p0new & ~p1new      # swap: slot0 <- parent1 (old)
                cmask = p0new & p1new       # both parents new
            else:
                bmask = np.zeros(len(s), bool)
                cmask = p0new
            apure = ~p0new
            rd = read[R + s]
            swap[s[bmask]] = True
            chunks.append((s[apure & rd], s[cmask & rd], s[bmask & rd],
                           s[bmask & ~rd], s[apure & ~rd], s[cmask & ~rd]))
        # uniform per-chunk start offsets so C nodes (new slot-0 parent) sit
        # at the same slots on every core and spoil at most one column
        starts = []
        off = 0
        for ci in range(6):
            w = max(len(ch[ci]) for ch in chunks)
            starts.append(off)
            off += w
            if ci == 2:                      # after Br: round AG span
                agb = (off + AGPAD - 1) // AGPAD * AGPAD
                off = agb
        cmid = starts[2]                     # A-weights span: Ar+Cr
        bend = starts[4]                     # B-weights span: Br+Bu
        npad = max(AGPAD, (off + AGPAD - 1) // AGPAD * AGPAD)
        k = (npad + P - 1) // P
        if use_swap:
            ranges = []
            if cmid > 0:
                ranges.append((0, 0, cmid))
            if bend > cmid:
                ranges.append((1, cmid, bend))
            if npad > bend:
                ranges.append((0, bend, npad))
        else:
            ranges = [(0, 0, npad)]
        core_slots = []
        for c in range(NCORES):
            ids = np.concatenate(chunks[c])
            sl = np.concatenate([st + np.arange(len(ch))
                                 for st, ch in zip(starts, chunks[c])]
                                ) if len(ids) else np.zeros(0, np.int64)
            core_slots.append((ids, sl.astype(np.int64)))
            pos[R + ids[read[R + ids]]] = blk + c * agb + sl[read[R + ids]]
        # dma_gather (int16-indexed, transposing) eligibility per parent slot
        prev_blk = blk_starts[-2] if len(blk_starts) > 1 else R
        g0 = "col"
        g1 = "col"
        cspans = []
        if USE_DG and prev_blk <= 32767:
            if use_swap:
                g0 = "dg-early"     # non-C slot0 rows are old; C fixed late
                cspans = [(starts[1], cmid), (starts[5], npad)]
            else:
                g0 = "dg-late"
        if use_swap:
            g1b = prev_blk          # slot1 rows all in prev level's block
            if USE_DG and blk - prev_blk <= 32767:
                g1 = "dg-late"
        else:
            g1b = 0
            if USE_DG and blk <= 32767:
                g1 = "dg-late"
        if use_swap:
            g1 = "col"              # keep pipelined per-column path
        fix0 = sorted({j for a, b in cspans if b > a
                       for j in range(a // P, (b + P - 1) // P)})
        specs.append({"k": k, "npad": npad, "agb": agb, "ranges": ranges,
                      "g0": g0, "g1": g1, "g1b": g1b, "fix0": fix0})
        shards.append(core_slots)
        blk += NCORES * agb

    K = sum(s["k"] for s in specs)
    # int16 gather-index columns for dma_gather slots
    G16 = sum((8 * s["k"] if s["g0"].startswith("dg") else 0) +
              (8 * s["k"] if s["g1"].startswith("dg") else 0) for s in specs)
    G16 = max(G16, 16)
    # gidx layout per level: [slot0 cols (k) | slot1 cols (k)]
    gidx = np.zeros((NCORES, P, 2 * K), np.int32)
    gidx16 = np.zeros((NCORES, P, G16), np.int16)
    src_rows = [[] for _ in range(NCORES)]
    dst_rows = [[] for _ in range(NCORES)]
    goff = 0
    g16off = 0
    soff = 0

    def wrap16(v):
        # index i -> partition i % 16, column i // 16; replicated to 128 parts
        return np.tile(v.reshape(-1, 16).T, (8, 1))

    for l0 in range(L):
        spec = specs[l0]
        k = spec["k"]
        nidx = k * P
        colmax = np.zeros(2 * k, np.int64)
        pa_all = np.zeros((NCORES, nidx), np.int64)
        pb_all = np.zeros((NCORES, nidx), np.int64)
        for c in range(NCORES):
            ids, sl = shards[l0][c]
            if len(ids):
                sw = swap[ids]
                pa = np.where(sw, idx[ids, 1], idx[ids, 0])   # slot0 parent
                pb = np.where(sw, idx[ids, 0], idx[ids, 1])   # slot1 parent
                pp = sl % P
                jj = sl // P
                gidx[c, pp, goff + jj] = pos[pa]
                gidx[c, pp, goff + k + jj] = pos[pb]
                pa_all[c, sl] = pos[pa]
                pb_all[c, sl] = pos[pb]
                np.maximum.at(colmax, jj, pos[pa])
                np.maximum.at(colmax, k + jj, pos[pb])
                src_rows[c].append(soff + sl)
                dst_rows[c].append(R + ids)
        prev_blk = blk_starts[l0 - 1] if l0 > 0 else R
        spec["early"] = [bool(colmax[col] < prev_blk) for col in range(2 * k)]
        spec["early_bound"] = int(prev_blk)
        if spec["g0"].startswith("dg"):
            spec["g0_off"] = g16off
            for c in range(NCORES):
                v = pa_all[c].copy()
                if spec["g0"] == "dg-early":
                    v[v >= prev_blk] = 0          # C slots fixed up late
                assert v.max() <= 32767
                gidx16[c, :, g16off: g16off + nidx // 16] = wrap16(v)
            g16off += nidx // 16
        if spec["g1"].startswith("dg"):
            spec["g1_off"] = g16off
            for c in range(NCORES):
                v = pb_all[c] - spec["g1b"]
                v[pb_all[c] == 0] = 0             # padding slots
                assert v.min() >= 0 and v.max() <= 32767
                gidx16[c, :, g16off: g16off + nidx // 16] = wrap16(v)
            g16off += nidx // 16
        goff += 2 * k
        soff += spec["npad"]
    src_rows = [np.concatenate(o) if o else np.zeros(0, np.int64) for o in src_rows]
    dst_rows = [np.concatenate(o) if o else np.zeros(0, np.int64) for o in dst_rows]
    return {
        "specs": specs,
        "slots": blk,
        "K": K,
        "G16": G16,
        "sum_npad": soff,
        "gidx": gidx,
        "gidx16": gidx16,
        "src_rows": src_rows,
        "dst_rows": dst_rows,
    }


# ---------------------------------------------------------------------------
# Bass program
# ---------------------------------------------------------------------------

def _build_program(specs, slots, K, G16, sum_npad):
    import concourse.bass as bass
    import concourse.tile as tile
    from concourse import bacc, library_config, mybir
    from concourse.masks import make_identity

    F16 = mybir.dt.float16
    F32 = mybir.dt.float32
    I32 = mybir.dt.int32
    AF = mybir.ActivationFunctionType

    nc = bacc.Bacc("TRN2", target_bir_lowering=False, debug=False,
                   num_devices=NCORES)
    # w1 blocks: 0 = own type, 1 = own type with parent halves swapped
    t_root = nc.dram_tensor("roots", [R, D], F16, kind="ExternalInput")
    t_w1 = nc.dram_tensor("w1", [P, 2 * 2048], F16, kind="ExternalInput")
    t_w2 = nc.dram_tensor("w2", [P, 1024], F16, kind="ExternalInput")
    t_b1 = nc.dram_tensor("b1", [P, 4], F32, kind="ExternalInput")
    t_b2 = nc.dram_tensor("b2", [P, D], F32, kind="ExternalInput")
    I16 = mybir.dt.int16
    t_gidx = nc.dram_tensor("gidx", [P, 2 * K], I32, kind="ExternalInput")
    t_gidx16 = nc.dram_tensor("gidx16", [P, G16], I16, kind="ExternalInput")
    t_sout = nc.dram_tensor("shard_out", [sum_npad, D], F16,
                            kind="ExternalOutput")
    buffer = nc.dram_tensor("buffer", [slots, D], F16, kind="Internal",
                            addr_space="Shared")
    groups = [list(range(NCORES))]
    k_max = max(s["k"] for s in specs)

    with tile.TileContext(nc) as tc:
        with (
            tc.tile_pool(name="const", bufs=1) as constp,
            tc.tile_pool(name="sbuf", bufs=2) as sbufp,
            tc.tile_pool(name="psum", bufs=1, space="PSUM") as psump,
            tc.tile_pool(name="dram", bufs=2, space="DRAM") as dramp,
        ):
            # dma_gather (InstDMAGatherAnt) runs from the gpsimd mlp library
            nc.gpsimd.load_library(library_config.mlp)
            ident = constp.tile([P, P], F16)
            make_identity(nc, ident[:])
            w1_sb = constp.tile([P, 2 * 2048], F16)
            nc.sync.dma_start(w1_sb[:], t_w1[:])
            w2_sb = constp.tile([P, 1024], F16)
            nc.sync.dma_start(w2_sb[:], t_w2[:])
            b1_sb = constp.tile([P, 4], F32)
            nc.sync.dma_start(b1_sb[:], t_b1[:])
            b2_sb = constp.tile([P, D], F32)
            nc.sync.dma_start(b2_sb[:], t_b2[:])
            gidx_sb = constp.tile([P, 2 * K], I32)
            nc.sync.dma_start(gidx_sb[:], t_gidx[:])
            gidx16_sb = constp.tile([P, G16], I16)
            nc.sync.dma_start(gidx16_sb[:], t_gidx16[:])

            # roots -> buffer[0:R]
            stg = sbufp.tile([P, (R // P) * D], F16, tag="stg")
            nc.sync.dma_start(
                stg[:], t_root[:].rearrange("(j p) d -> p j d", p=P))
            nc.sync.dma_start(
                buffer[0:R, :].rearrange("(j p) d -> p j d", p=P), stg[:])

            L = len(specs)
            goffs = [0] * (L + 1)
            for l in range(L):
                goffs[l + 1] = goffs[l] + 2 * specs[l]["k"]
            soffs = [0] * (L + 1)
            for l in range(L):
                soffs[l + 1] = soffs[l] + specs[l]["npad"]
            blks = [R]
            for l in range(L):
                blks.append(blks[l] + NCORES * specs[l]["agb"])

            gx_t = {}
            xg_t = {}

            def get_xg(l):
                if l not in xg_t:
                    k = specs[l]["k"]
                    xg_t[l] = [sbufp.tile([P, 2, P * k], F16, tag=f"xg{s}",
                                          name=f"xg{s}") for s in range(2)]
                return xg_t[l]

            def slot_cols(l, s, phase):
                # int32-gather columns of parent slot s to emit in this phase
                spec = specs[l]
                k = spec["k"]
                g = spec["g0"] if s == 0 else spec["g1"]
                if g == "col":
                    return [j for j in range(k)
                            if spec["early"][s * k + j] == (phase == "early")]
                if s == 0 and g == "dg-early" and phase == "late":
                    return spec["fix0"]      # C columns redone after the AG
                return []

            def emit_gathers(l, phase):
                spec = specs[l]
                k = spec["k"]
                xg = get_xg(l)
                bound = spec["early_bound"]
                goff = goffs[l]
                for s in range(2):
                    g = spec["g0"] if s == 0 else spec["g1"]
                    if g.startswith("dg"):
                        if (g == "dg-early") != (phase == "early"):
                            continue
                        base = spec["g1b"] if s == 1 else 0
                        hi = bound if g == "dg-early" else blks[l]
                        off = spec["g0_off"] if s == 0 else spec["g1_off"]
                        nc.gpsimd.dma_gather(
                            xg[s][:], buffer[base:hi, :],
                            gidx16_sb[:, off: off + k * P // 16],
                            k * P, k * P, D, transpose=True)
                cols0 = slot_cols(l, 0, phase)
                cols1 = slot_cols(l, 1, phase)
                if not (cols0 or cols1):
                    return
                if l not in gx_t:
                    gx_t[l] = sbufp.tile([P, 2 * k_max * D], F16, tag="gx",
                                         name="gx")[:, : 2 * k * D]
                gx = gx_t[l]
                src = buffer[0:bound, :] if phase == "early" else buffer[:]
                for s, cols in ((0, cols0), (1, cols1)):
                    for j in cols:
                        col = s * k + j
                        nc.gpsimd.indirect_dma_start(
                            out=gx[:, col * D:(col + 1) * D], out_offset=None,
                            in_=src,
                            in_offset=bass.IndirectOffsetOnAxis(
                                ap=gidx_sb[:, goff + col: goff + col + 1],
                                axis=0))

            def emit_transposes(l, phase):
                spec = specs[l]
                k = spec["k"]
                xg = get_xg(l)
                for s in range(2):
                    cols = slot_cols(l, s, phase)
                    if not cols:
                        continue
                    gx = gx_t[l]
                    for j in cols:
                        col = s * k + j
                        for h in range(2):
                            tp = psump.tile([P, P], F16, tag="tpose", bufs=3,
                                            name="tp")
                            nc.tensor.transpose(
                                tp[:],
                                gx[:, col * D + h * P: col * D + (h + 1) * P],
                                ident[:])
                            nc.vector.tensor_copy(
                                xg[s][:, h, j * P:(j + 1) * P], tp[:])

            for l, spec in enumerate(specs):
                k = spec["k"]
                npad = spec["npad"]
                agb = spec["agb"]
                soff = soffs[l]
                blk = blks[l]
                if l == 0:
                    emit_gathers(0, "early")
                    emit_gathers(0, "late")
                    emit_transposes(0, "early")
                    emit_transposes(0, "late")
                xg = xg_t.pop(l)
                gx_t.pop(l, None)

                h_sb = [sbufp.tile([P, P * k_max], F16, tag=f"h{oc}",
                                   name=f"h{oc}")[:, : npad] for oc in range(4)]
                e_sb = sbufp.tile([P, k_max * D], F16, tag="e",
                                  name="e")[:, : k * D]
                cc_in = dramp.tile([agb, D], F16, tag="cc",
                                   name="cc") if agb else None
                # phase split: the AllGather needs cols < agb; split the
                # natural 512-wide PSUM groups (not the columns) so the split
                # adds no extra weight reloads
                agb128 = min(npad, (agb + P - 1) // P * P)
                l1groups = []
                for wb, r0, r1 in spec["ranges"]:
                    for g0 in range(r0, r1, PSUM_N):
                        l1groups.append((wb, g0, min(g0 + PSUM_N, r1)))

                def emit_l1(groups):
                    # layer 1: h = gelu(x @ W1 + b1), feature-major
                    for wb, g0, g1 in groups:
                        ng = g1 - g0
                        cols = slice(g0, g1)
                        for oc in range(4):
                            hp = psump.tile([P, PSUM_N], F32,
                                            tag=f"hp{oc % 2}",
                                            name="hp")[:, :ng]
                            for ic in range(4):
                                w = w1_sb[:, wb * 2048 + ic * 512 + oc * P:
                                          wb * 2048 + ic * 512 + (oc + 1) * P]
                                nc.tensor.matmul(
                                    hp, lhsT=w,
                                    rhs=xg[ic // 2][:, ic % 2, cols],
                                    start=(ic == 0), stop=(ic == 3))
                            nc.scalar.activation(
                                h_sb[oc][:, cols], hp, AF.Gelu,
                                bias=b1_sb[:, oc: oc + 1])

                def emit_l2(j0, j1):
                    # layer 2: emb = h @ W2 + b2, node-major via lhsT = h
                    for j in range(j0, j1):
                        w = min(P, npad - j * P)
                        ep = psump.tile([P, D], F32, tag=f"ep{j % 2}",
                                        name="ep")[:w, :]
                        for ic in range(4):
                            nc.tensor.matmul(
                                ep, lhsT=h_sb[ic][:, j * P: j * P + w],
                                rhs=w2_sb[:, ic * D:(ic + 1) * D],
                                start=(ic == 0), stop=(ic == 3))
                        nc.vector.tensor_add(
                            e_sb[:w, j * D:(j + 1) * D], ep, b2_sb[:w, :])
                        # cc DMAs alone on the sync queue gate the AllGather;
                        # shard_out streaming rides the scalar HWDGE queue
                        nc.scalar.dma_start(
                            t_sout[soff + j * P: soff + j * P + w, :],
                            e_sb[:w, j * D:(j + 1) * D])
                        wc = min(agb - j * P, w)
                        if wc > 0:
                            nc.sync.dma_start(
                                cc_in[j * P: j * P + wc, :],
                                e_sb[:wc, j * D:(j + 1) * D])

                ga = [g for g in l1groups if g[1] < agb128]
                gd = [g for g in l1groups if g[1] >= agb128]
                t_split = min(k, (agb + P - 1) // P)
                emit_l1(ga)
                emit_l2(0, t_split)
                # next level's AG-independent gathers + transposes and this
                # level's unread-row compute run in the AllGather's shadow
                if l + 1 < L:
                    emit_gathers(l + 1, "early")
                    emit_transposes(l + 1, "early")
                if agb:
                    nc.gpsimd.collective_compute(
                        "AllGather", mybir.AluOpType.bypass,
                        replica_groups=groups,
                        ins=[cc_in[:]],
                        outs=[buffer[blk: blk + NCORES * agb, :]])
                emit_l1(gd)
                emit_l2(t_split, k)
                if l + 1 < L:
                    emit_gathers(l + 1, "late")
                    emit_transposes(l + 1, "late")
    nc.compile()
    return nc


# ---------------------------------------------------------------------------
# entry point
# ---------------------------------------------------------------------------

_CACHE = {}


def _get_program(key, *args):
    if key not in _CACHE:
        _CACHE[key] = _build_program(*args)
    return _CACHE[key]


def kernel(root_node_embeddings, enc_W1, enc_b1, enc_W2, enc_b2,
           trunk_node_inputs_indices, trunk_node_types):
    from concourse import bass_utils

    root = np.asarray(root_node_embeddings, dtype=np.float32)
    W1 = np.asarray(enc_W1, dtype=np.float32)
    W2 = np.asarray(enc_W2, dtype=np.float32)
    b1 = np.asarray(enc_b1, dtype=np.float32)
    b2 = np.asarray(enc_b2, dtype=np.float32)
    idx = np.asarray(trunk_node_inputs_indices)
    types = np.asarray(trunk_node_types)
    if types.ndim > 1:
        types = types[:, 0]
    types = types.astype(np.int64)
    idx64 = idx.astype(np.int64)
    n = idx64.shape[0]

    plan = _plan(idx64, types)
    specs = plan["specs"]
    key = (tuple((s["k"], s["npad"], s["agb"], tuple(s["ranges"]),
                  tuple(s["early"]), s["g0"], s["g1"], tuple(s["fix0"]))
                 for s in specs), plan["slots"])
    nc = _get_program(key, specs, plan["slots"], plan["K"], plan["G16"],
                      plan["sum_npad"])

    def w1tab(t):
        # [128, 2*2048] fp16: block0 = W1[t], block1 = parent-swapped W1[t]
        wn = W1[t]
        ws = np.concatenate([wn[D:], wn[:D]], 0)
        blocks = [w.reshape(4, P, 4, P).transpose(1, 0, 2, 3).reshape(P, -1)
                  for w in (wn, ws)]
        return np.ascontiguousarray(np.concatenate(blocks, 1), dtype=np.float16)

    def w2tab(t):
        # [128, 4*256] fp16: block ic = W2[t][ic*128:(ic+1)*128, :]
        return np.ascontiguousarray(
            W2[t].reshape(4, P, D).transpose(1, 0, 2).reshape(P, -1),
            dtype=np.float16)

    in_maps = []
    for c in range(NCORES):
        t = c // 2
        in_maps.append({
            "roots": np.ascontiguousarray(root, dtype=np.float16),
            "w1": w1tab(t),
            "w2": w2tab(t),
            "b1": np.ascontiguousarray(b1[t].reshape(4, P).T,
                                       dtype=np.float32),
            "b2": np.ascontiguousarray(np.tile(b2[t], (P, 1)),
                                       dtype=np.float32),
            "gidx": np.ascontiguousarray(plan["gidx"][c]),
            "gidx16": np.ascontiguousarray(plan["gidx16"][c]),
        })

    res = bass_utils.run_bass_kernel_spmd(
        nc, in_maps, core_ids=list(range(NCORES)),
        trace=bool(int(os.environ.get("DAG_KERNEL_TRACE", "0"))))
    if res.exec_time_ns is not None:
        kernel.last_exec_time_ns = res.exec_time_ns

    out = np.zeros((R + n, D), np.float32)
    out[:R] = root
    for c in range(NCORES):
        dst = plan["dst_rows"][c]
        if len(dst):
            out[dst] = res.results[c]["shard_out"][plan["src_rows"][c]].astype(
                np.float32)
    return out


kernel.last_exec_time_ns = None


# revision 30
# speedup vs baseline: 1.0081x; 1.0081x over previous
"""Trainium2 Bass kernel for the DagnabbitAutoEncoder DAG scan.

Strategy: level-scheduled scan (node depth = 1 + max parent depth; ~28
levels), type-sharded across the 8 NeuronCores (type t -> cores 2t, 2t+1),
fp16 datapath with fp32 PSUM accumulation.

v2 improvements over the AllGather-per-level baseline:
  * parent-swap: every node has >=1 parent in the immediately previous
    level.  Nodes whose OTHER parent is old get their parent slots swapped
    (using a row-permuted copy of W1) so gather slot 0 always reads an old
    row.  All slot-0 gather columns + their PE transposes are then
    independent of the previous AllGather and run in its shadow.
  * read-packing: rows never referenced by any later node are placed after
    the AllGather span and are not exchanged at all; together with 32-row
    padding this cuts AllGather bytes ~40%.
  * the second MLP layer runs with lhsT = h (feature-major hidden) and
    rhs = W2, producing the node-major output tile directly in PSUM - no
    output PE transposes, and the PSUM->SBUF eviction fuses the b2 add.
  * gather columns are grouped [slot0 | slot1] per level and issued as a
    few fused multi-column indirect DMAs instead of one DMA per column.
  * per-tile output DMAs into the collective staging buffer overlap the
    remaining compute; the last level (no future readers) skips its
    AllGather entirely.

The host merges the 8 partial outputs (each core owns its shard rows).
"""

import math
import os

import numpy as np

R = 256
D = 256
NCORES = 8
P = 128
PSUM_N = 512
AGPAD = 16      # AllGather span padding granularity (rows)
SWAP_MIN = int(os.environ.get("DAG_SWAP_MIN", "2048"))
USE_DG = int(os.environ.get("DAG_USE_DG", "0"))


# ---------------------------------------------------------------------------
# host-side preprocessing
# ---------------------------------------------------------------------------

def _compute_levels(idx):
    n = idx.shape[0]
    depth = np.zeros(R + n, np.int32)
    ia = idx[:, 0]
    ib = idx[:, 1]
    d = depth
    for i in range(n):
        da = d[ia[i]]
        db = d[ib[i]]
        d[R + i] = (da if da > db else db) + 1
    return depth[R:]


def _plan(idx, types):
    n = idx.shape[0]
    lv = _compute_levels(idx)
    L = int(lv.max()) if n else 0
    rowlv = np.zeros(R + n, np.int64)
    rowlv[R:] = lv
    read = np.zeros(R + n, bool)
    read[idx.ravel()] = True

    order = np.argsort(lv, kind="stable")
    lv_sorted = lv[order]
    level_nodes = []
    lo = 0
    for l in range(1, L + 1):
        hi = lo + np.searchsorted(lv_sorted[lo:], l + 1)
        level_nodes.append(order[lo:hi])
        lo = hi

    pos = np.zeros(R + n, np.int64)
    pos[:R] = np.arange(R)
    swap = np.zeros(n, bool)

    blk = R
    blk_starts = []
    specs = []       # per level: k, npad, agb, ranges
    shards = []      # per level: per-core (node_ids, slots)
    for l0 in range(L):
        l = l0 + 1
        nodes = level_nodes[l0]
        blk_starts.append(blk)
        per_core = []
        for t in range(4):
            nt = nodes[types[nodes] == t]
            per_core.append(nt[0::2])
            per_core.append(nt[1::2])
        # the parent-swap trick pays for its extra weight ranges only on
        # levels with substantial per-core shards
        use_swap = len(nodes) > SWAP_MIN
        chunks = []   # per core: (Ar, Cr, Br, Bu, Au, Cu) node-id arrays
        for c in range(NCORES):
            s = per_core[c]
            if l > 1:
                p0new = rowlv[idx[s, 0]] == l - 1
                p1new = rowlv[idx[s, 1]] == l - 1
            else:
                p0new = np.zeros(len(s), bool)
                p1new = np.zeros(len(s), bool)
            if use_swap:
                bmask = p0new & ~p1new      # swap: slot0 <- parent1 (old)
                cmask = p0new & p1new       # both parents new
            else:
                bmask = np.zeros(len(s), bool)
                cmask = p0new
            apure = ~p0new
            rd = read[R + s]
            swap[s[bmask]] = True
            chunks.append((s[apure & rd], s[cmask & rd], s[bmask & rd],
                           s[bmask & ~rd], s[apure & ~rd], s[cmask & ~rd]))
        # uniform per-chunk start offsets so C nodes (new slot-0 parent) sit
        # at the same slots on every core and spoil at most one column
        starts = []
        off = 0
        for ci in range(6):
            w = max(len(ch[ci]) for ch in chunks)
            starts.append(off)
            off += w
            if ci == 2:                      # after Br: round AG span
                agb = (off + AGPAD - 1) // AGPAD * AGPAD
                off = agb
        cmid = starts[2]                     # A-weights span: Ar+Cr
        bend = starts[4]                     # B-weights span: Br+Bu
        npad = max(AGPAD, (off + AGPAD - 1) // AGPAD * AGPAD)
        k = (npad + P - 1) // P
        if use_swap:
            ranges = []
            if cmid > 0:
                ranges.append((0, 0, cmid))
            if bend > cmid:
                ranges.append((1, cmid, bend))
            if npad > bend:
                ranges.append((0, bend, npad))
        else:
            ranges = [(0, 0, npad)]
        core_slots = []
        for c in range(NCORES):
            ids = np.concatenate(chunks[c])
            sl = np.concatenate([st + np.arange(len(ch))
                                 for st, ch in zip(starts, chunks[c])]
                                ) if len(ids) else np.zeros(0, np.int64)
            core_slots.append((ids, sl.astype(np.int64)))
            pos[R + ids[read[R + ids]]] = blk + c * agb + sl[read[R + ids]]
        # dma_gather (int16-indexed, transposing) eligibility per parent slot
        prev_blk = blk_starts[-2] if len(blk_starts) > 1 else R
        g0 = "col"
        g1 = "col"
        cspans = []
        if USE_DG and prev_blk <= 32767:
            if use_swap:
                g0 = "dg-early"     # non-C slot0 rows are old; C fixed late
                cspans = [(starts[1], cmid), (starts[5], npad)]
            else:
                g0 = "dg-late"
        if use_swap:
            g1b = prev_blk          # slot1 rows all in prev level's block
            if USE_DG and blk - prev_blk <= 32767:
                g1 = "dg-late"
        else:
            g1b = 0
            if USE_DG and blk <= 32767:
                g1 = "dg-late"
        if use_swap:
            g1 = "col"              # keep pipelined per-column path
        fix0 = sorted({j for a, b in cspans if b > a
                       for j in range(a // P, (b + P - 1) // P)})
        specs.append({"k": k, "npad": npad, "agb": agb, "ranges": ranges,
                      "g0": g0, "g1": g1, "g1b": g1b, "fix0": fix0})
        shards.append(core_slots)
        blk += NCORES * agb

    K = sum(s["k"] for s in specs)
    # int16 gather-index columns for dma_gather slots
    G16 = sum((8 * s["k"] if s["g0"].startswith("dg") else 0) +
              (8 * s["k"] if s["g1"].startswith("dg") else 0) for s in specs)
    G16 = max(G16, 16)
    # gidx layout per level: [slot0 cols (k) | slot1 cols (k)]
    gidx = np.zeros((NCORES, P, 2 * K), np.int32)
    gidx16 = np.zeros((NCORES, P, G16), np.int16)
    src_rows = [[] for _ in range(NCORES)]
    dst_rows = [[] for _ in range(NCORES)]
    goff = 0
    g16off = 0
    soff = 0

    def wrap16(v):
        # index i -> partition i % 16, column i // 16; replicated to 128 parts
        return np.tile(v.reshape(-1, 16).T, (8, 1))

    for l0 in range(L):
        spec = specs[l0]
        k = spec["k"]
        nidx = k * P
        colmax = np.zeros(2 * k, np.int64)
        pa_all = np.zeros((NCORES, nidx), np.int64)
        pb_all = np.zeros((NCORES, nidx), np.int64)
        for c in range(NCORES):
            ids, sl = shards[l0][c]
            if len(ids):
                sw = swap[ids]
                pa = np.where(sw, idx[ids, 1], idx[ids, 0])   # slot0 parent
                pb = np.where(sw, idx[ids, 0], idx[ids, 1])   # slot1 parent
                pp = sl % P
                jj = sl // P
                gidx[c, pp, goff + jj] = pos[pa]
                gidx[c, pp, goff + k + jj] = pos[pb]
                pa_all[c, sl] = pos[pa]
                pb_all[c, sl] = pos[pb]
                np.maximum.at(colmax, jj, pos[pa])
                np.maximum.at(colmax, k + jj, pos[pb])
                src_rows[c].append(soff + sl)
                dst_rows[c].append(R + ids)
        prev_blk = blk_starts[l0 - 1] if l0 > 0 else R
        spec["early"] = [bool(colmax[col] < prev_blk) for col in range(2 * k)]
        spec["early_bound"] = int(prev_blk)
        if spec["g0"].startswith("dg"):
            spec["g0_off"] = g16off
            for c in range(NCORES):
                v = pa_all[c].copy()
                if spec["g0"] == "dg-early":
                    v[v >= prev_blk] = 0          # C slots fixed up late
                assert v.max() <= 32767
                gidx16[c, :, g16off: g16off + nidx // 16] = wrap16(v)
            g16off += nidx // 16
        if spec["g1"].startswith("dg"):
            spec["g1_off"] = g16off
            for c in range(NCORES):
                v = pb_all[c] - spec["g1b"]
                v[pb_all[c] == 0] = 0             # padding slots
                assert v.min() >= 0 and v.max() <= 32767
                gidx16[c, :, g16off: g16off + nidx // 16] = wrap16(v)
            g16off += nidx // 16
        goff += 2 * k
        soff += spec["npad"]
    src_rows = [np.concatenate(o) if o else np.zeros(0, np.int64) for o in src_rows]
    dst_rows = [np.concatenate(o) if o else np.zeros(0, np.int64) for o in dst_rows]
    return {
        "specs": specs,
        "slots": blk,
        "K": K,
        "G16": G16,
        "sum_npad": soff,
        "gidx": gidx,
        "gidx16": gidx16,
        "src_rows": src_rows,
        "dst_rows": dst_rows,
    }


# ---------------------------------------------------------------------------
# Bass program
# ---------------------------------------------------------------------------

def _build_program(specs, slots, K, G16, sum_npad):
    import concourse.bass as bass
    import concourse.tile as tile
    from concourse import bacc, library_config, mybir
    from concourse.masks import make_identity

    F16 = mybir.dt.float16
    F32 = mybir.dt.float32
    I32 = mybir.dt.int32
    AF = mybir.ActivationFunctionType

    nc = bacc.Bacc("TRN2", target_bir_lowering=False, debug=False,
                   num_devices=NCORES)
    # w1 blocks: 0 = own type, 1 = own type with parent halves swapped
    t_root = nc.dram_tensor("roots", [R, D], F16, kind="ExternalInput")
    t_w1 = nc.dram_tensor("w1", [P, 2 * 2048], F16, kind="ExternalInput")
    t_w2 = nc.dram_tensor("w2", [P, 1024], F16, kind="ExternalInput")
    t_b1 = nc.dram_tensor("b1", [P, 4], F32, kind="ExternalInput")
    t_b2 = nc.dram_tensor("b2", [P, D], F32, kind="ExternalInput")
    I16 = mybir.dt.int16
    t_gidx = nc.dram_tensor("gidx", [P, 2 * K], I32, kind="ExternalInput")
    t_gidx16 = nc.dram_tensor("gidx16", [P, G16], I16, kind="ExternalInput")
    t_sout = nc.dram_tensor("shard_out", [sum_npad, D], F16,
                            kind="ExternalOutput")
    buffer = nc.dram_tensor("buffer", [slots, D], F16, kind="Internal",
                            addr_space="Shared")
    groups = [list(range(NCORES))]
    k_max = max(s["k"] for s in specs)

    with tile.TileContext(nc) as tc:
        with (
            tc.tile_pool(name="const", bufs=1) as constp,
            tc.tile_pool(name="sbuf", bufs=2) as sbufp,
            tc.tile_pool(name="psum", bufs=1, space="PSUM") as psump,
            tc.tile_pool(name="dram", bufs=2, space="DRAM") as dramp,
        ):
            # dma_gather (InstDMAGatherAnt) runs from the gpsimd mlp library
            nc.gpsimd.load_library(library_config.mlp)
            ident = constp.tile([P, P], F16)
            make_identity(nc, ident[:])
            w1_sb = constp.tile([P, 2 * 2048], F16)
            nc.sync.dma_start(w1_sb[:], t_w1[:])
            w2_sb = constp.tile([P, 1024], F16)
            nc.sync.dma_start(w2_sb[:], t_w2[:])
            b1_sb = constp.tile([P, 4], F32)
            nc.sync.dma_start(b1_sb[:], t_b1[:])
            b2_sb = constp.tile([P, D], F32)
            nc.sync.dma_start(b2_sb[:], t_b2[:])
            gidx_sb = constp.tile([P, 2 * K], I32)
            nc.sync.dma_start(gidx_sb[:], t_gidx[:])
            gidx16_sb = constp.tile([P, G16], I16)
            nc.sync.dma_start(gidx16_sb[:], t_gidx16[:])

            # roots -> buffer[0:R]
            stg = sbufp.tile([P, (R // P) * D], F16, tag="stg")
            nc.sync.dma_start(
                stg[:], t_root[:].rearrange("(j p) d -> p j d", p=P))
            nc.sync.dma_start(
                buffer[0:R, :].rearrange("(j p) d -> p j d", p=P), stg[:])

            L = len(specs)
            goffs = [0] * (L + 1)
            for l in range(L):
                goffs[l + 1] = goffs[l] + 2 * specs[l]["k"]
            soffs = [0] * (L + 1)
            for l in range(L):
                soffs[l + 1] = soffs[l] + specs[l]["npad"]
            blks = [R]
            for l in range(L):
                blks.append(blks[l] + NCORES * specs[l]["agb"])

            gx_t = {}
            xg_t = {}

            def get_xg(l):
                if l not in xg_t:
                    k = specs[l]["k"]
                    xg_t[l] = [sbufp.tile([P, 2, P * k], F16, tag=f"xg{s}",
                                          name=f"xg{s}") for s in range(2)]
                return xg_t[l]

            def slot_cols(l, s, phase):
                # int32-gather columns of parent slot s to emit in this phase
                spec = specs[l]
                k = spec["k"]
                g = spec["g0"] if s == 0 else spec["g1"]
                if g == "col":
                    return [j for j in range(k)
                            if spec["early"][s * k + j] == (phase == "early")]
                if s == 0 and g == "dg-early" and phase == "late":
                    return spec["fix0"]      # C columns redone after the AG
                return []

            def emit_gathers(l, phase):
                spec = specs[l]
                k = spec["k"]
                xg = get_xg(l)
                bound = spec["early_bound"]
                goff = goffs[l]
                for s in range(2):
                    g = spec["g0"] if s == 0 else spec["g1"]
                    if g.startswith("dg"):
                        if (g == "dg-early") != (phase == "early"):
                            continue
                        base = spec["g1b"] if s == 1 else 0
                        hi = bound if g == "dg-early" else blks[l]
                        off = spec["g0_off"] if s == 0 else spec["g1_off"]
                        nc.gpsimd.dma_gather(
                            xg[s][:], buffer[base:hi, :],
                            gidx16_sb[:, off: off + k * P // 16],
                            k * P, k * P, D, transpose=True)
                cols0 = slot_cols(l, 0, phase)
                cols1 = slot_cols(l, 1, phase)
                if not (cols0 or cols1):
                    return
                if l not in gx_t:
                    gx_t[l] = sbufp.tile([P, 2 * k_max * D], F16, tag="gx",
                                         name="gx")[:, : 2 * k * D]
                gx = gx_t[l]
                src = buffer[0:bound, :] if phase == "early" else buffer[:]
                for s, cols in ((0, cols0), (1, cols1)):
                    for j in cols:
                        col = s * k + j
                        nc.gpsimd.indirect_dma_start(
                            out=gx[:, col * D:(col + 1) * D], out_offset=None,
                            in_=src,
                            in_offset=bass.IndirectOffsetOnAxis(
                                ap=gidx_sb[:, goff + col: goff + col + 1],
                                axis=0))

            def emit_transposes(l, phase):
                spec = specs[l]
                k = spec["k"]
                xg = get_xg(l)
                for s in range(2):
                    cols = slot_cols(l, s, phase)
                    if not cols:
                        continue
                    gx = gx_t[l]
                    for j in cols:
                        col = s * k + j
                        for h in range(2):
                            tp = psump.tile([P, P], F16, tag="tpose", bufs=3,
                                            name="tp")
                            nc.tensor.transpose(
                                tp[:],
                                gx[:, col * D + h * P: col * D + (h + 1) * P],
                                ident[:])
                            nc.vector.tensor_copy(
                                xg[s][:, h, j * P:(j + 1) * P], tp[:])

            for l, spec in enumerate(specs):
                k = spec["k"]
                npad = spec["npad"]
                agb = spec["agb"]
                soff = soffs[l]
                blk = blks[l]
                if l == 0:
                    emit_gathers(0, "early")
                    emit_gathers(0, "late")
                    emit_transposes(0, "early")
                    emit_transposes(0, "late")
                xg = xg_t.pop(l)
                gx_t.pop(l, None)

                h_sb = [sbufp.tile([P, P * k_max], F16, tag=f"h{oc}",
                                   name=f"h{oc}")[:, : npad] for oc in range(4)]
                e_sb = sbufp.tile([P, k_max * D], F16, tag="e",
                                  name="e")[:, : k * D]
                cc_in = dramp.tile([agb, D], F16, tag="cc",
                                   name="cc") if agb else None
                # phase split: the AllGather needs cols < agb; split the
                # natural 512-wide PSUM groups (not the columns) so the split
                # adds no extra weight reloads
                agb128 = min(npad, (agb + P - 1) // P * P)
                l1groups = []
                for wb, r0, r1 in spec["ranges"]:
                    for g0 in range(r0, r1, PSUM_N):
                        l1groups.append((wb, g0, min(g0 + PSUM_N, r1)))

                def emit_l1(groups):
                    # layer 1: h = gelu(x @ W1 + b1), feature-major
                    for wb, g0, g1 in groups:
                        ng = g1 - g0
                        cols = slice(g0, g1)
                        for oc in range(4):
                            hp = psump.tile([P, PSUM_N], F32,
                                            tag=f"hp{oc % 2}",
                                            name="hp")[:, :ng]
                            for ic in range(4):
                                w = w1_sb[:, wb * 2048 + ic * 512 + oc * P:
                                          wb * 2048 + ic * 512 + (oc + 1) * P]
                                nc.tensor.matmul(
                                    hp, lhsT=w,
                                    rhs=xg[ic // 2][:, ic % 2, cols],
                                    start=(ic == 0), stop=(ic == 3))
                            nc.scalar.activation(
                                h_sb[oc][:, cols], hp, AF.Gelu,
                                bias=b1_sb[:, oc: oc + 1])

                def emit_l2(j0, j1):
                    # layer 2: emb = h @ W2 + b2, node-major via lhsT = h
                    for j in range(j0, j1):
                        w = min(P, npad - j * P)
                        ep = psump.tile([P, D], F32, tag=f"ep{j % 2}",
                                        name="ep")[:w, :]
                        for ic in range(4):
                            nc.tensor.matmul(
                                ep, lhsT=h_sb[ic][:, j * P: j * P + w],
                                rhs=w2_sb[:, ic * D:(ic + 1) * D],
                                start=(ic == 0), stop=(ic == 3))
                        nc.vector.tensor_add(
                            e_sb[:w, j * D:(j + 1) * D], ep, b2_sb[:w, :])
                        # cc DMAs alone on the sync queue gate the AllGather;
                        # shard_out streaming rides the scalar HWDGE queue
                        nc.scalar.dma_start(
                            t_sout[soff + j * P: soff + j * P + w, :],
                            e_sb[:w, j * D:(j + 1) * D])
                        wc = min(agb - j * P, w)
                        if wc > 0:
                            nc.sync.dma_start(
                                cc_in[j * P: j * P + wc, :],
                                e_sb[:wc, j * D:(j + 1) * D])

                ga = [g for g in l1groups if g[1] < agb128]
                gd = [g for g in l1groups if g[1] >= agb128]
                t_split = min(k, (agb + P - 1) // P)
                emit_l1(ga)
                emit_l2(0, t_split)
                # next level's AG-independent gathers + transposes and this
                # level's unread-row compute run in the AllGather's shadow
                if l + 1 < L:
                    emit_gathers(l + 1, "early")
                    emit_transposes(l + 1, "early")
                if agb:
                    nc.gpsimd.collective_compute(
                        "AllGather", mybir.AluOpType.bypass,
                        replica_groups=groups,
                        ins=[cc_in[:]],
                        outs=[buffer[blk: blk + NCORES * agb, :]])
                emit_l1(gd)
                emit_l2(t_split, k)
                if l + 1 < L:
                    emit_gathers(l + 1, "late")
                    emit_transposes(l + 1, "late")
    nc.compile()
    return nc


# ---------------------------------------------------------------------------
# entry point
# ---------------------------------------------------------------------------

_CACHE = {}


def _get_program(key, *args):
    if key not in _CACHE:
        _CACHE[key] = _build_program(*args)
    return _CACHE[key]


def kernel(root_node_embeddings, enc_W1, enc_b1, enc_W2, enc_b2,
           trunk_node_inputs_indices, trunk_node_types):
    from concourse import bass_utils

    root = np.asarray(root_node_embeddings, dtype=np.float32)
    W1 = np.asarray(enc_W1, dtype=np.float32)
    W2 = np.asarray(enc_W2, dtype=np.float32)
    b1 = np.asarray(enc_b1, dtype=np.float32)
    b2 = np.asarray(enc_b2, dtype=np.float32)
    idx = np.asarray(trunk_node_inputs_indices)
    types = np.asarray(trunk_node_types)
    if types.ndim > 1:
        types = types[:, 0]
    types = types.astype(np.int64)
    idx64 = idx.astype(np.int64)
    n = idx64.shape[0]

    plan = _plan(idx64, types)
    specs = plan["specs"]
    key = (tuple((s["k"], s["npad"], s["agb"], tuple(s["ranges"]),
                  tuple(s["early"]), s["g0"], s["g1"], tuple(s["fix0"]))
                 for s in specs), plan["slots"])
    nc = _get_program(key, specs, plan["slots"], plan["K"], plan["G16"],
                      plan["sum_npad"])

    def w1tab(t):
        # [128, 2*2048] fp16: block0 = W1[t], block1 = parent-swapped W1[t]
        wn = W1[t]
        ws = np.concatenate([wn[D:], wn[:D]], 0)
        blocks = [w.reshape(4, P, 4, P).transpose(1, 0, 2, 3).reshape(P, -1)
                  for w in (wn, ws)]
        return np.ascontiguousarray(np.concatenate(blocks, 1), dtype=np.float16)

    def w2tab(t):
        # [128, 4*256] fp16: block ic = W2[t][ic*128:(ic+1)*128, :]
        return np.ascontiguousarray(
            W2[t].reshape(4, P, D).transpose(1, 0, 2).reshape(P, -1),
            dtype=np.float16)

    in_maps = []
    for c in range(NCORES):
        t = c // 2
        in_maps.append({
            "roots": np.ascontiguousarray(root, dtype=np.float16),
            "w1": w1tab(t),
            "w2": w2tab(t),
            "b1": np.ascontiguousarray(b1[t].reshape(4, P).T,
                                       dtype=np.float32),
            "b2": np.ascontiguousarray(np.tile(b2[t], (P, 1)),
                                       dtype=np.float32),
            "gidx": np.ascontiguousarray(plan["gidx"][c]),
            "gidx16": np.ascontiguousarray(plan["gidx16"][c]),
        })

    res = bass_utils.run_bass_kernel_spmd(
        nc, in_maps, core_ids=list(range(NCORES)),
        trace=bool(int(os.environ.get("DAG_KERNEL_TRACE", "0"))))
    if res.exec_time_ns is not None:
        kernel.last_exec_time_ns = res.exec_time_ns

    out = np.zeros((R + n, D), np.float32)
    out[:R] = root
    for c in range(NCORES):
        dst = plan["dst_rows"][c]
        if len(dst):
            out[dst] = res.results[c]["shard_out"][plan["src_rows"][c]].astype(
                np.float32)
    return out


kernel.last_exec_time_ns = None
